# revision 33
# baseline (speedup 1.0000x reference)
"""AdaptiveConv3 Trainium2 kernel.

Full model: 7-layer conv generator (3x3, BN folded on host) -> per-pixel
3x3x6 adaptive kernels (einsum over fixed basis) -> per-pixel contraction
with unfolded input patches.

Sharding: data-parallel over batch N=8, one image per NeuronCore.

Per-core pipeline (image [64, 128, 128]):
  - conv generator stack on TensorE in fp16, channel-major, row-stacked dual
    buffers so vertical tap pairs contract K=128 (6 matmul streams per tile
    vs 9); tap-major emission over band groups shares LDWEIGHTS and keeps
    the PE HAM-warm.
  - the basis contraction is refactored: zb[(l,c)] = x conv basis_l (6 fixed
    depthwise 3x3 convs) runs on TensorE as dense matmuls, so the per-pixel
    stage contracts over l=6 instead of the 9 taps
    (out[c,m,p] = sum_l gen[m,l,p] * zb[c,l,p]).
  - per-pixel stage on VectorE in pixel-major layout (one image row = 128
    partitions): per basis index l, ONE tensor_tensor multiply covers all 6
    m-blocks at FD=384 using free-dim broadcast APs
    (genT[p,m,l] bcast over c  *  zbT[p,l,c] bcast over m), followed by
    dense FD=384 adds; gen/zb rows are transposed to pixel-major on TensorE.
  - output written pixel-major [HW, (m,c)]; host reorders to NCHW.
"""

from contextlib import ExitStack

import numpy as np

N, C, H, W = 8, 64, 128, 128
INTER = 64
FEAT = 6
M = 6
KS = 3
L = KS * KS          # 9
NMID = 5
GOUT = FEAT * M      # 36
KCH = M * L          # 54 kern channels (m*9+d)
OUTC = C * M         # 384
HP, WP = H + 2, W + 2          # 130
NPAD = HP * WP                 # 16900
HWTOT = H * W                  # 16384
BN_EPS = 1e-5
NBAND = 32                     # 4-row bands
RPB = 4                        # rows per band
NT = RPB * W                   # 512 free elems per conv tile

_CACHE = {}


def _build_program(split_gps=2):
    """Build the SPMD Bass program (same graph on all 8 cores).

    split_gps: number of m-blocks (of 6) whose einsum2 MACs run on gpsimd
    instead of the vector engine (load balancing between the two).
    """
    import concourse.bacc as bacc
    import concourse.mybir as mybir
    from concourse.tile import TileContext

    fp32 = mybir.dt.float32
    bf16 = mybir.dt.float16
    AF = mybir.ActivationFunctionType
    OP = mybir.AluOpType

    nc = bacc.Bacc("TRN2", debug=False)

    # ---------------- DRAM I/O ----------------
    x_d = nc.dram_tensor("x", [C, H, W], fp32, kind="ExternalInput")
    # paired stationaries, host layout [128 (=2 ky x 64 ic), 7*3*64]
    wpair_d = nc.dram_tensor("wpair", [128, 7 * 3 * 64], bf16, kind="ExternalInput")
    # single (ky=2) stationaries [64 ic, 7*3*64]
    wsing_d = nc.dram_tensor("wsing", [64, 7 * 3 * 64], bf16, kind="ExternalInput")
    bias_d = nc.dram_tensor("bias", [64, 7], fp32, kind="ExternalInput")
    zwpair_d = nc.dram_tensor("zwpair", [128, 9 * 128], bf16, kind="ExternalInput")
    zwsing_d = nc.dram_tensor("zwsing", [64, 9 * 128], bf16, kind="ExternalInput")
    ident_d = nc.dram_tensor("ident", [128, 128], bf16, kind="ExternalInput")
    out_d = nc.dram_tensor("out", [HWTOT, OUTC], bf16, kind="ExternalOutput")

    with TileContext(nc) as tc, ExitStack() as es:
        # ------------- persistent SBUF -------------
        x2 = nc.alloc_sbuf_tensor("x2", [128, NPAD], bf16)
        t2a = nc.alloc_sbuf_tensor("t2a", [128, NPAD], bf16)
        t2b = nc.alloc_sbuf_tensor("t2b", [128, NPAD], bf16)
        wpair_sb = nc.alloc_sbuf_tensor("wpair_sb", [128, 7 * 3 * 64], bf16)
        wsing_sb = nc.alloc_sbuf_tensor("wsing_sb", [64, 7 * 3 * 64], bf16)
        bias_sb = nc.alloc_sbuf_tensor("bias_sb", [64, 7], fp32)
        zwpair_sb = nc.alloc_sbuf_tensor("zwpair_sb", [128, 9 * 128], bf16)
        zwsing_sb = nc.alloc_sbuf_tensor("zwsing_sb", [64, 9 * 128], bf16)
        ident_sb = nc.alloc_sbuf_tensor("ident_sb", [128, 128], bf16)
        gen_sb = nc.alloc_sbuf_tensor("gen_sb", [GOUT, HWTOT], bf16)

        x2v = x2[:].rearrange("p (h w) -> p h w", h=HP, w=WP)
        t2av = t2a[:].rearrange("p (h w) -> p h w", h=HP, w=WP)
        t2bv = t2b[:].rearrange("p (h w) -> p h w", h=HP, w=WP)

        # ------------- load constants -------------
        # Only the pad borders need zeroing (interiors are fully written):
        # top/bottom pad rows, left/right pad columns, and the bottom-half's
        # two trailing rows (its interior covers padded rows 0..127 only).
        for bufv in (x2v, t2av, t2bv):
            nc.vector.memset(bufv[:, 0:1, :], 0.0)       # padded row 0
            nc.vector.memset(bufv[:, 129:130, :], 0.0)   # padded row 129
            nc.vector.memset(bufv[64:128, 128:129, :], 0.0)  # bottom-half row 128
            nc.vector.memset(bufv[:, :, 0:1], 0.0)       # left pad col
            nc.vector.memset(bufv[:, :, 129:130], 0.0)   # right pad col
        # x (f32 -> bf16): top half holds padded image at rows 1..128;
        # bottom half (partitions 64:128) the same image one padded row up,
        # so partition 64+c at padded row r equals partition c at row r+1.
        for c0 in range(0, H, 16):
            nc.gpsimd.dma_start(out=x2v[0:64, 1 + c0:17 + c0, 1:129],
                                in_=x_d[:, c0:c0 + 16, :])
            nc.gpsimd.dma_start(out=x2v[64:128, c0:c0 + 16, 1:129],
                                in_=x_d[:, c0:c0 + 16, :])
        nc.sync.dma_start(out=wpair_sb[:], in_=wpair_d[:])
        nc.sync.dma_start(out=wsing_sb[:], in_=wsing_d[:])
        nc.sync.dma_start(out=bias_sb[:], in_=bias_d[:])
        nc.sync.dma_start(out=zwpair_sb[:], in_=zwpair_d[:])
        nc.sync.dma_start(out=zwsing_sb[:], in_=zwsing_d[:])
        nc.sync.dma_start(out=ident_sb[:], in_=ident_d[:])

        # ------------- tile pools -------------
        conv_ps = es.enter_context(tc.tile_pool(name="conv_ps", bufs=2, space="PSUM"))
        zb_ps_pool = es.enter_context(tc.tile_pool(name="zb_ps", bufs=2, space="PSUM"))
        kt_ps_pool = es.enter_context(tc.tile_pool(name="kt_ps", bufs=2, space="PSUM"))
        sl_ps_pool = es.enter_context(tc.tile_pool(name="sl_ps", bufs=2, space="PSUM"))
        zb_pool = es.enter_context(tc.tile_pool(name="zb", bufs=3))
        zbt_pool = es.enter_context(tc.tile_pool(name="zbt", bufs=4))
        kt_pool = es.enter_context(tc.tile_pool(name="kt", bufs=3))
        acc_pool = es.enter_context(tc.tile_pool(name="acc", bufs=4))
        pl_pool = es.enter_context(tc.tile_pool(name="pl", bufs=4))

        layer_src = [x2v, t2av, t2bv, t2av, t2bv, t2av, t2bv]
        layer_dst = [t2av, t2bv, t2av, t2bv, t2av, t2bv, None]

        def conv_group(lyr, grp, gsz):
            """Conv tiles for `gsz` bands, tap-major so each stationary is
            loaded once per group (LDWEIGHTS amortized, dense PE burst)."""
            src = layer_src[lyr]
            bands = [grp * gsz + i for i in range(gsz)]
            tiles = [conv_ps.tile([64, NT], fp32, tag="conv",
                                  name=f"cps_{lyr}_{b}") for b in bands]
            views = [t[:].rearrange("p (h w) -> p h w", h=RPB, w=W) for t in tiles]
            # paired taps: ky in {0,1}, K=128
            for kx in range(3):
                off = (lyr * 3 + kx) * 64
                for b, psv in zip(bands, views):
                    r0 = b * RPB
                    nc.tensor.matmul(psv, wpair_sb[:, off:off + 64],
                                     src[:, r0:r0 + RPB, kx:kx + W],
                                     start=(kx == 0), stop=False)
            # single taps: ky=2, K=64 (top half only)
            for kx in range(3):
                off = (lyr * 3 + kx) * 64
                for b, psv in zip(bands, views):
                    r0 = b * RPB
                    nc.tensor.matmul(psv, wsing_sb[:, off:off + 64],
                                     src[0:64, r0 + 2:r0 + 2 + RPB, kx:kx + W],
                                     start=False, stop=(kx == 2))
            for b, ps, psv in zip(bands, tiles, views):
                r0 = b * RPB
                if lyr < 6:
                    dst = layer_dst[lyr]
                    func = AF.Tanh if lyr == 0 else AF.Identity
                    top = dst[0:64, r0 + 1:r0 + 1 + RPB, 1:1 + W]
                    nc.scalar.activation(top, psv, func,
                                         bias=bias_sb[:, lyr:lyr + 1], scale=1.0)
                    bot = dst[64:128, r0:r0 + RPB, 1:1 + W]
                    nc.sync.dma_start(out=bot, in_=top)
                else:
                    # final layer -> gen (36 channels, tanh, unpadded)
                    dst = gen_sb[0:GOUT, b * NT:(b + 1) * NT]
                    nc.scalar.activation(dst, ps[0:GOUT, :], AF.Tanh,
                                         bias=bias_sb[0:GOUT, 6:7], scale=1.0)

        def zb_band(band):
            """6 fixed basis depthwise convs of x for one 4-row band,
            channel layout (l,c) in 3 col-groups of 128."""
            r0 = band * RPB
            tiles = []
            for g in range(3):
                ps = zb_ps_pool.tile([128, NT], fp32, tag="zbps",
                                     name=f"zbps_{band}_{g}")
                psv = ps[:].rearrange("p (h w) -> p h w", h=RPB, w=W)
                for kx in range(3):
                    off = (g * 3 + kx) * 128
                    nc.tensor.matmul(psv, zwpair_sb[:, off:off + 128],
                                     x2v[:, r0:r0 + RPB, kx:kx + W],
                                     start=(kx == 0), stop=False)
                for kx in range(3):
                    off = (g * 3 + kx) * 128
                    nc.tensor.matmul(psv, zwsing_sb[:, off:off + 128],
                                     x2v[0:64, r0 + 2:r0 + 2 + RPB, kx:kx + W],
                                     start=False, stop=(kx == 2))
                t = zb_pool.tile([128, NT], bf16, tag=f"zb{g}",
                                 name=f"zb_{band}_{g}")
                nc.scalar.activation(t[:], ps[:], AF.Copy)
                tiles.append(t)
            zb_tiles[band] = tiles

        zb_tiles = {}

        def row_einsum2(r):
            band, sub = r // RPB, r % RPB
            kt_ps = kt_ps_pool.tile([128, GOUT], bf16, tag="ktps")
            nc.tensor.transpose(kt_ps[:], gen_sb[:, r * W:(r + 1) * W],
                                ident_sb[0:GOUT, 0:GOUT])
            kt = kt_pool.tile([128, GOUT], bf16, tag="kt")
            nc.scalar.activation(kt[:], kt_ps[:], AF.Copy)
            zbt = []
            for g in range(3):
                ps = sl_ps_pool.tile([128, 128], bf16, tag="slps",
                                     name=f"zbtps_{r}_{g}")
                nc.tensor.transpose(
                    ps[:], zb_tiles[band][g][:, sub * W:(sub + 1) * W],
                    ident_sb[:])
                t = zbt_pool.tile([128, 128], bf16, tag=f"zbt{g}",
                                  name=f"zbt_{r}_{g}")
                nc.scalar.activation(t[:], ps[:], AF.Copy)
                zbt.append(t)
            acc = acc_pool.tile([128, OUTC], bf16, tag="acc")
            accv = acc[:].rearrange("p (m c) -> p m c", c=64)
            kt3 = kt[:].rearrange("p (m l) -> p m l", l=M)
            # one tensor_tensor per l covering all 6 m-blocks (FD=384) via
            # free-dim broadcast APs; then dense FD=384 adds.
            for l in range(M):
                g, lh = l // 2, l % 2
                gbc = kt3[:, :, l:l + 1].to_broadcast((128, M, 64))
                zbc = zbt[g][:, lh * 64:(lh + 1) * 64].unsqueeze(1)\
                    .to_broadcast((128, M, 64))
                if l == 0:
                    nc.vector.tensor_tensor(accv, zbc, gbc, op=OP.mult)
                else:
                    pl = pl_pool.tile([128, OUTC], bf16, tag="pl")
                    plv = pl[:].rearrange("p (m c) -> p m c", c=64)
                    nc.vector.tensor_tensor(plv, zbc, gbc, op=OP.mult)
                    nc.vector.tensor_tensor(acc[:], acc[:], pl[:], op=OP.add)
            nc.sync.dma_start(out=out_d[r * W:(r + 1) * W, :], in_=acc[:])

        # ------------- wavefront emission (2-band groups) -------------
        GSZ = 2
        NGRP = NBAND // GSZ
        for step in range(NGRP + 10):
            for lyr in range(7):
                grp = step - lyr
                if 0 <= grp < NGRP:
                    conv_group(lyr, grp, GSZ)
            gz = step - 7
            if 0 <= gz < NGRP:
                for band in range(gz * GSZ, (gz + 1) * GSZ):
                    zb_band(band)
            ge = step - 8
            if 0 <= ge < NGRP:
                for r in range(ge * GSZ * RPB, (ge + 1) * GSZ * RPB):
                    row_einsum2(r)

    nc.finalize()
    return nc


def _prep_inputs(inputs):
    """Host-side weight prep: BN folding, tap pairing, bases selector."""
    bf = np.float16

    f = lambda k: np.asarray(inputs[k], np.float32)
    W0, b0, g0, be0, m0, v0 = (f(k) for k in ("W0", "b0", "g0", "be0", "m0", "v0"))
    Wmid, bmid = f("Wmid"), f("bmid")
    Wf, bf_, gf, bef, mf, vf = (f(k) for k in ("Wf", "bf", "gf", "bef", "mf", "vf"))
    bases = f("bases")

    s0 = g0 / np.sqrt(v0 + BN_EPS)
    W0p = W0 * s0[:, None, None, None]
    b0p = (b0 - m0) * s0 + be0
    sf = gf / np.sqrt(vf + BN_EPS)
    Wfp = Wf * sf[:, None, None, None]
    bfp = (bf_ - mf) * sf + bef

    # layer weights [oc, ic, ky, kx] -> paired/single stationaries
    Wf64 = np.zeros((64, 64, 3, 3), np.float32)
    Wf64[:GOUT] = Wfp
    Ws = [W0p] + [Wmid[i] for i in range(NMID)] + [Wf64]
    wpair = np.zeros((7, 3, 128, 64), np.float32)
    wsing = np.zeros((7, 3, 64, 64), np.float32)
    for lyr in range(7):
        w = Ws[lyr]
        for kx in range(3):
            wpair[lyr, kx, 0:64] = w[:, :, 0, kx].T     # ky=0 -> top partitions
            wpair[lyr, kx, 64:128] = w[:, :, 1, kx].T   # ky=1 -> bottom
            wsing[lyr, kx] = w[:, :, 2, kx].T           # ky=2

    bias = np.zeros((64, 7), np.float32)
    bias[:, 0] = b0p
    for i in range(NMID):
        bias[:, 1 + i] = bmid[i]
    bias[:GOUT, 6] = bfp

    # zb (basis depthwise conv) stationaries: col-group g holds channels
    # (l, c) for l in {2g, 2g+1}; value = bases[l, ky*3+kx] on the diagonal.
    zwpair = np.zeros((3, 3, 128, 128), np.float32)
    zwsing = np.zeros((3, 3, 64, 128), np.float32)
    eye = np.eye(64, dtype=np.float32)
    for g in range(3):
        for kx in range(3):
            for lh in range(2):
                l = 2 * g + lh
                for ky in range(2):
                    zwpair[g, kx, ky * 64:(ky + 1) * 64, lh * 64:(lh + 1) * 64] = \
                        eye * bases[l, ky * 3 + kx]
                zwsing[g, kx, :, lh * 64:(lh + 1) * 64] = eye * bases[l, 6 + kx]
    zwpair = np.ascontiguousarray(zwpair.transpose(2, 0, 1, 3)).reshape(128, 9 * 128)
    zwsing = np.ascontiguousarray(zwsing.transpose(2, 0, 1, 3)).reshape(64, 9 * 128)

    ident = np.eye(128, dtype=np.float32)
    wpair = wpair.transpose(2, 0, 1, 3).reshape(128, 7 * 3 * 64)
    wsing = wsing.transpose(2, 0, 1, 3).reshape(64, 7 * 3 * 64)
    return {
        "wpair": np.ascontiguousarray(wpair).astype(bf),
        "wsing": np.ascontiguousarray(wsing).astype(bf),
        "bias": bias,
        "zwpair": zwpair.astype(bf),
        "zwsing": zwsing.astype(bf),
        "ident": ident.astype(bf),
    }


def _env_int(name, default):
    import os
    v = os.environ.get(name)
    return default if v is None else int(v)


def _install_ntff_hook():
    """Provide antenv.axon_hooks (missing in this image) so bass_utils can
    NTFF-profile under axon via the injected libaxon_pjrt.so."""
    import sys
    import types
    if "antenv.axon_hooks" in sys.modules:
        return
    try:
        import antenv
        from trn_agent_boot.trn_boot import _ntff_profile_via_ctypes
        hook = _ntff_profile_via_ctypes("/opt/axon/libaxon_pjrt.so")
    except Exception:
        return
    mod = types.ModuleType("antenv.axon_hooks")
    holder = {"h": hook}
    mod.set_axon_ntff_profile_hook = lambda h: holder.__setitem__("h", h)
    mod.get_axon_ntff_profile_hook = lambda: holder.get("h")
    sys.modules["antenv.axon_hooks"] = mod
    antenv.axon_hooks = mod


def kernel(**inputs):
    from concourse import bass_utils

    split_gps = _env_int("ADAPT_SPLIT_GPS", 0)
    key = ("prog", split_gps)
    if key not in _CACHE:
        _CACHE[key] = _build_program(split_gps)
    nc = _CACHE[key]

    shared = _prep_inputs(inputs)
    x_full = np.asarray(inputs["input"], np.float32)
    in_maps = [dict(shared, x=np.ascontiguousarray(x_full[i])) for i in range(N)]

    trace = bool(_env_int("ADAPT_TRACE", 0))
    if trace:
        _install_ntff_hook()
    res = bass_utils.run_bass_kernel_spmd(
        nc, in_maps, core_ids=list(range(N)), trace=trace)
    if trace:
        _CACHE["last_result"] = res

    out = np.empty((N, OUTC, H, W), np.float32)
    for i in range(N):
        o = np.asarray(res.results[i]["out"], dtype=np.float32)  # [HW, (m,c)]
        o4 = o.reshape(H, W, M, C)
        out[i] = o4.transpose(3, 2, 0, 1).reshape(OUTC, H, W)
    return out


if __name__ == "__main__":
    import time
    t0 = time.time()
    nc = _build_program()
    print(f"program built in {time.time() - t0:.1f}s")
